# revision 21
# baseline (speedup 1.0000x reference)
"""Trainium2 Bass kernel for the HH neuron network problem.

Math (validated bit-exact vs reference in numpy):
  The [N,N] synaptic matrices only enter through per-type masked column sums
  cols[t,j] = sum_i W[i,j]*(ty[i,j]==t+1), and the synaptic gating s decays
  independently of V (s_k = s0 * decay^(k+1)).  The per-step coupling collapses
  to a scalar:
      c_k = dot(AK[k], V_k) - corr[k]
      AK[k,j] = sum_t wk[k,t] * a[t,j],   a = s0 * cols
      wk[k,t] = G_SYN*SIGN[t]*decay_t^(k+1) (f32-iterated),
      corr[k] = sum_t wk[k,t]*E_SYN[t]*sum_j a[t,j]
  after which each step is elementwise HH gating + V update + clip.

Distribution (8 cores):
  Launch A: column-shard W/types [N, N/8] per core -> cols_shard [6, N/8].
            (the per-type current sums' all-reduce becomes a host gather of
             the tiny a-vector between launches)
  Host:     assemble a, wk, corr (all tiny, f32).
  Launch B: replicated scan on every core (no per-step collectives; a
            per-step 20us all-reduce floor x1000 steps would dominate).
            Per-step dot vectors AK live in SBUF, computed on-device by a
            [1000,6]x[6,4096] matmul.  Each core writes an interleaved
            1/8 column slice of V_hist selected by partition_id.
"""

import sys
import numpy as np

for p in ("/opt/trn_rl_repo", "/opt/trn_rl_repo/concourse"):
    if p not in sys.path:
        sys.path.insert(0, p)

import concourse.bass as bass
import concourse.bacc as bacc
import concourse.mybir as mybir
from concourse.bass_utils import run_bass_kernel_spmd
from concourse.tile import TileContext
from concourse.mybir import AluOpType as Op, ActivationFunctionType as Act


def _register_hh_ops():
    """Register the fused custom DVE ops for the HH scan (idempotent).

    HH_D    : d = in0*in1 - 1              (denominators; block-2 sign folded)
    HH_GMP  : w = in0*(1 - in1)            (gating G*(1-S))
    HH_CUBE : out = in0^3*in1*imm2         (m^3*h*(-DT*G_NA))
    HH_POW4 : out = in0^4*imm2             (n^4*(-DT*G_K))
    HH_FIN2 : out = nan_to_num(clip(in0+in1+s0, -s1, s1))   (V update tail)
    """
    import concourse.dve_ops as dve_ops
    from concourse.dve_spec import (
        Spec, Src0, Src1, C0, C1, C2, Zero, One, eq, select, minn, maxx, lower,
        sq,
    )
    from concourse.dve_ops import has_src1

    if "HH_D" in dve_ops._SUB_OPCODE_FOR_NAME:
        return {o.name: o for o in dve_ops.OPS if o.name.startswith("HH_")}

    t2 = Src0 + Src1 + C0
    specs = {
        "HH_D": Spec(
            body=Src0 * Src1 - One,
            reference=lambda in0, in1, s0, s1, imm2:
                (in0 * in1.reshape(in0.shape) - np.float32(1.0)).astype(np.float32)),
        "HH_GMP": Spec(
            body=Src0 * (One - Src1),
            reference=lambda in0, in1, s0, s1, imm2:
                (in0 * (np.float32(1.0) - in1.reshape(in0.shape))).astype(np.float32)),
        "HH_CUBE": Spec(
            body=sq(Src0) * Src0 * Src1 * C2,
            reference=lambda in0, in1, s0, s1, imm2:
                (in0 * in0 * in0 * in1.reshape(in0.shape) * np.float32(imm2)).astype(np.float32)),
        "HH_POW4": Spec(
            body=sq(sq(Src0)) * C2,
            reference=lambda in0, in1, s0, s1, imm2:
                ((in0 * in0) * (in0 * in0) * np.float32(imm2)).astype(np.float32)),
        "HH_FIN2": Spec(
            body=select(eq(t2, t2), maxx(minn(t2, C1), Zero - C1), Zero),
            reference=lambda in0, in1, s0, s1, imm2: np.where(
                np.isnan(in0 + in1.reshape(in0.shape) + s0), 0.0,
                np.clip(in0 + in1.reshape(in0.shape) + s0, -s1, s1)).astype(np.float32)),
    }
    ops = {}
    for name, sp in specs.items():
        op = dve_ops.DveOp(name, sp, subdim=False, uops_sha={})
        dve_ops.OPS.append(op)
        dve_ops._SUB_OPCODE_FOR_NAME[name] = (
            dve_ops._CUSTOM_DVE_ROW_BASE + len(dve_ops.OPS) - 1
        )
        dve_ops.CUSTOM_DVE_SPECS[name] = sp
        for ver in ("v3", "v4"):
            r = dve_ops.DveOpSpec(
                name=name, opcode=dve_ops.get_dve_sub_opcode(name),
                uops=lower(sp, ver=ver), rd1_en=has_src1(sp),
            )
            op.uops_sha[ver] = r.sha(ver)
        ops[name] = op
    return ops


F32 = mybir.dt.float32
I32 = mybir.dt.int32

N = 4096
NCORES = 8
NSH = N // NCORES          # 512 columns per core
P = 128                    # partitions
Q = N // P                 # 32 free elements per partition
QSH = Q // NCORES          # 4 free columns per core output slice

DT = np.float32(0.01)
G_NA, G_K, G_L = np.float32(120.0), np.float32(36.0), np.float32(0.3)
E_L = np.float32(-54.387)
G_SYN = np.float32(0.1)
E_SYN = np.array([0.0, -70.0, -90.0, 0.0, 0.0, 0.0], np.float32)
TAU = np.array([2.0, 5.0, 10.0, 100.0, 50.0, 30.0], np.float32)
SIGN = np.array([-1.0, 1.0, 1.0, -1.0, -1.0, -1.0], np.float32)

E15 = float(np.float32(np.exp(1.5)))
E2 = float(np.float32(np.exp(2.0)))
LN_007 = float(np.log(0.07))
LN_4 = float(np.log(4.0))
LN_0125 = float(np.log(0.125))
C1 = float(np.float32(1.0) - DT * G_L)         # V leak factor
CNA = float(DT * G_NA)
CK = float(DT * G_K)
# DT-prescaled rate constants (A~ = DT*A, B~ = DT*B)
LN_007DT = float(np.log(np.float32(0.07) * DT))
LN_4DT = float(np.log(np.float32(4.0) * DT))
LN_0125DT = float(np.log(np.float32(0.125) * DT))
FDT = float(DT)


# ---------------------------------------------------------------- launch A --
def build_cols_kernel(reps=1):
    """Per-core: W [N, NSH], ty [N, NSH] -> cols [6, NSH] masked column sums."""
    nc = bacc.Bacc("TRN2", num_devices=NCORES)
    w_in = nc.declare_dram_parameter("w", [N, NSH], F32, isOutput=False)
    ty_in = nc.declare_dram_parameter("ty", [N, NSH], I32, isOutput=False)
    cols_out = nc.declare_dram_parameter("cols", [6, NSH], F32, isOutput=True)

    ntiles = N // P  # 32 row tiles of [128, NSH]
    with TileContext(nc) as tc:
        with (
            tc.tile_pool(name="io", bufs=4) as io,
            tc.tile_pool(name="msk", bufs=4) as mskp,
            tc.tile_pool(name="const", bufs=1) as constp,
            tc.tile_pool(name="ps", bufs=1, space="PSUM") as ps,
        ):
            ones = constp.tile([P, 1], F32)
            nc.vector.memset(ones[:], 1.0)
            accs = [ps.tile([1, NSH], F32, tag=f"acc{t}", name=f"acc{t}") for t in range(6)]
            for i in range(ntiles * reps):
                i = i % ntiles
                wt = io.tile([P, NSH], F32, tag="wt")
                tt = io.tile([P, NSH], I32, tag="tt")
                nc.sync.dma_start(out=wt[:], in_=w_in[i * P:(i + 1) * P, :])
                nc.sync.dma_start(out=tt[:], in_=ty_in[i * P:(i + 1) * P, :])
                for t in range(1, 7):
                    mk = mskp.tile([P, NSH], F32, tag=f"mk{t}")
                    eng = nc.vector
                    eng.scalar_tensor_tensor(
                        mk[:], tt[:], float(t), wt[:], Op.is_equal, Op.mult
                    )
                    nc.tensor.matmul(
                        accs[t - 1][:], ones[:], mk[:],
                        start=(i == 0), stop=(i == ntiles - 1),
                        skip_group_check=True,
                    )
            for t in range(6):
                csb = constp.tile([1, NSH], F32, tag=f"csb{t}", name=f"csb{t}")
                nc.scalar.copy(csb[:], accs[t][:])
                nc.sync.dma_start(out=cols_out[t:t + 1, :], in_=csb[:])
    return nc


# ---------------------------------------------------------------- launch B --
def build_scan_kernel(num_steps, reps=1):
    """Replicated HH scan.  Inputs (identical on all cores):
         a6   [6, N]      a = s0*cols (natural j order)
         wkT  [6, num_steps]
         corr [1, num_steps]
         ie2  [P, Q]      DT*I_ext + DT*G_L*E_L  (device layout)
         v0   [P, Q]      initial V (device layout)
         g0   [P, 3*Q]    initial [m|h|n] (device layout)
       Output per core: vh [num_steps, NSH] - column slice q in [4c, 4c+4).
    """
    nc = bacc.Bacc("TRN2", num_devices=NCORES)
    a_in = nc.declare_dram_parameter("a6", [6, N], F32, isOutput=False)
    wk_in = nc.declare_dram_parameter("wkT", [6, num_steps], F32, isOutput=False)
    corr_in = nc.declare_dram_parameter("corr", [1, num_steps], F32, isOutput=False)
    ie2_in = nc.declare_dram_parameter("ie2", [P, Q], F32, isOutput=False)
    v0_in = nc.declare_dram_parameter("v0", [P, Q], F32, isOutput=False)
    g0_in = nc.declare_dram_parameter("g0", [P, 3 * Q], F32, isOutput=False)
    vh_out = nc.declare_dram_parameter("vh", [num_steps, NSH], F32, isOutput=True)

    KC = 500 if num_steps % 500 == 0 else num_steps  # GEMM k-chunk
    nkc = num_steps // KC
    hh = _register_hh_ops()

    with TileContext(nc) as tc:
        with (
            tc.tile_pool(name="big", bufs=1) as big,
            tc.tile_pool(name="state", bufs=2) as st,
            tc.tile_pool(name="scr", bufs=2) as scr,
            tc.tile_pool(name="ps", bufs=2, space="PSUM") as ps,
            tc.tile_pool(name="psck", bufs=2, space="PSUM") as psck,
        ):
            # ---- persistent/SBUF-resident data ----
            ak = big.tile([P, num_steps, Q], F32)       # per-step dot vectors
            a6 = big.tile([6, N], F32)
            wkT = big.tile([6, num_steps], F32)
            corr = big.tile([1, num_steps], F32)
            ie2 = big.tile([P, Q], F32)
            ones = big.tile([P, P], F32)
            mones = big.tile([1, P], F32)
            cA = big.tile([P, 3, Q], F32)
            nt = big.tile([P, 3, Q], F32)               # numerators; block2 = 1
            b_ah = big.tile([P, 1], F32)
            b_bm = big.tile([P, 1], F32)
            b_bn = big.tile([P, 1], F32)
            b_25 = big.tile([P, 1], F32)
            b_01 = big.tile([P, 1], F32)
            nc.vector.memset(b_ah[:], LN_007DT)
            nc.vector.memset(b_bm[:], LN_4DT)
            nc.vector.memset(b_bn[:], LN_0125DT)
            nc.vector.memset(b_25[:], float(np.float32(2.5) * DT))
            nc.vector.memset(b_01[:], float(np.float32(0.1) * DT))
            nc.sync.dma_start(out=a6[:], in_=a_in[:, :])
            nc.sync.dma_start(out=wkT[:], in_=wk_in[:, :])
            nc.sync.dma_start(out=corr[:], in_=corr_in[:, :])
            nc.sync.dma_start(out=ie2[:], in_=ie2_in[:, :])
            # ones/mones carry the DT factor so ck_psum = DT*(dot - corr)
            nc.vector.memset(ones[:], float(DT))
            nc.vector.memset(mones[:], -float(DT))
            nc.vector.memset(cA[:, 0, :], E15)
            nc.vector.memset(cA[:, 1, :], 1.0)
            nc.vector.memset(cA[:, 2, :], -E2)
            nc.vector.memset(nt[:, 2, :], -FDT)

            # ---- phase 2: AK[p, k, q] = sum_t wk[k,t] * a[t, q*128+p] ----
            for jc in range(Q):
                for kc in range(nkc):
                    pt = ps.tile([P, KC], F32, tag="gemm")
                    nc.tensor.matmul(
                        pt[:], a6[:, jc * P:(jc + 1) * P],
                        wkT[:, kc * KC:(kc + 1) * KC],
                        start=True, stop=True,
                    )
                    dst = ak[:, kc * KC:(kc + 1) * KC, jc:jc + 1].squeeze(2)
                    if (jc + kc) % 2 == 0:
                        nc.scalar.copy(dst, pt[:])
                    else:
                        nc.vector.tensor_copy(dst, pt[:])

            # ---- phase 3: the scan ----
            for rep in range(reps):
              # initial state (re-loaded per rep; only the last rep's output
              # survives -- reps>1 is a pure-timing configuration)
              v = st.tile([P, Q], F32, tag="V")
              g = st.tile([P, 3, Q], F32, tag="G")
              nc.sync.dma_start(out=v[:], in_=v0_in[:, :])
              nc.sync.dma_start(out=g[:].rearrange("p a q -> p (a q)"), in_=g0_in[:, :])
              for k in range(num_steps):
                ab = scr.tile([P, 6, Q], F32, tag="AB")   # [am ah an bm bh bn]
                u = scr.tile([P, Q], F32, tag="u")
                d = scr.tile([P, 3, Q], F32, tag="D")
                pp = scr.tile([P, 1], F32, tag="pp")
                dotj = scr.tile([P, Q], F32, tag="dotj")
                zna = scr.tile([P, Q], F32, tag="zna")
                zk = scr.tile([P, Q], F32, tag="zk")
                fb = scr.tile([P, Q], F32, tag="fb")
                z1 = scr.tile([P, Q], F32, tag="z1")
                vn = st.tile([P, Q], F32, tag="V")
                gn = st.tile([P, 3, Q], F32, tag="G")
                ck = psck.tile([P, 1], F32, tag="ck")

                # coupling scalar: ck = DT*(sum_j AK[k,j]*V_j - corr_k)
                nc.vector.scalar_tensor_tensor(
                    dotj[:], ak[:, k, :], 1.0, v[:], Op.mult, Op.mult,
                    accum_out=pp[:],
                )
                nc.tensor.matmul(ck[:], ones[:], pp[:], start=True, stop=False,
                                 skip_group_check=True)
                nc.tensor.matmul(ck[:], mones[:], corr[0:1, k:k + 1],
                                 start=False, stop=True, skip_group_check=True)

                # rate functions (A~ = DT*A in AB blocks 0..2, B~ = DT*B in 3..5)
                nc.scalar.activation(u[:], v[:], Act.Exp, bias=1.0, scale=-0.1)
                nc.scalar.activation(ab[:, 1, :], v[:], Act.Exp, bias=b_ah[:], scale=-0.05)
                nc.scalar.activation(ab[:, 3, :], v[:], Act.Exp, bias=b_bm[:], scale=-1.0 / 18)
                nc.scalar.activation(ab[:, 5, :], v[:], Act.Exp, bias=b_bn[:], scale=-1.0 / 80)
                nc.scalar.activation(nt[:, 0, :], v[:], Act.Identity, bias=b_25[:], scale=float(-0.1 * DT))
                nc.scalar.activation(nt[:, 1, :], v[:], Act.Identity, bias=b_01[:], scale=float(-0.01 * DT))
                u_b = u[:].unsqueeze(1).to_broadcast((P, 3, Q))
                nc.vector._custom_dve(hh["HH_D"], out=d[:], in0=u_b, in1=cA[:])
                r = scr.tile([P, 3, Q], F32, tag="R")
                nc.vector.reciprocal_approx_fast(r[:], d[:])
                q_view = ab[:].rearrange("p (a b) q -> p a b q", a=3, b=2)[:, :, 0, :]
                nc.vector.tensor_tensor(q_view, nt[:], r[:], Op.mult)

                # gating: G' = (A~ - (A~+B~)*G) + G   (inf-inf -> NaN preserved,
                # matching the reference's Euler form)
                sS = scr.tile([P, 3, Q], F32, tag="S")
                w2 = scr.tile([P, 3, Q], F32, tag="W2")
                nc.gpsimd.tensor_tensor(sS[:], ab[:, 0:3, :], ab[:, 3:6, :], Op.add)
                nc.vector.tensor_tensor(w2[:], sS[:], g[:], Op.mult)
                nc.gpsimd.tensor_tensor(sS[:], ab[:, 0:3, :], w2[:], Op.subtract)
                nc.vector.tensor_tensor(gn[:], sS[:], g[:], Op.add)

                # currents + V update
                m3h = scr.tile([P, Q], F32, tag="m3h")
                n4 = scr.tile([P, Q], F32, tag="n4")
                nc.vector._custom_dve(hh["HH_CUBE"], out=m3h[:], in0=gn[:, 0, :],
                                      in1=gn[:, 1, :], imm2=-CNA)
                nc.vector._custom_dve(hh["HH_POW4"], out=n4[:], in0=gn[:, 2, :],
                                      imm2=-CK)
                nc.vector.scalar_tensor_tensor(zna[:], v[:], 50.0, m3h[:],
                                               Op.subtract, Op.mult)
                nc.vector.scalar_tensor_tensor(zk[:], v[:], -77.0, n4[:],
                                               Op.subtract, Op.mult)
                nc.vector.scalar_tensor_tensor(fb[:], v[:], C1, ie2[:],
                                               Op.mult, Op.add)
                nc.gpsimd.tensor_tensor(z1[:], zna[:], zk[:], Op.add)
                nc.vector._custom_dve(hh["HH_FIN2"], out=vn[:], in0=z1[:],
                                      in1=fb[:], s0=ck[:], s1=100.0)

                nc.sync.dma_start(out=vh_out[k, :], in_=vn[:, 0:QSH])
                v, g = vn, gn
    return nc


# ------------------------------------------------------------------- host --
def _host_prep(I_ext, V0, m0, h0, n0, s0, cols, num_steps, c):
    """Device data for core c with neuron q-blocks rotated by 4c, so the
    compile-time output slice cols [0:4) selects core c's global slice."""
    f32 = np.float32
    a = (np.asarray(s0, f32) * cols).astype(f32)                  # [6, N]
    decay = (f32(1.0) - DT / TAU).astype(f32)
    Dk = np.empty((num_steps, 6), f32)
    cur = np.ones(6, f32)
    for k in range(num_steps):
        cur = (cur * decay).astype(f32)
        Dk[k] = cur
    wk = (G_SYN * SIGN[None, :] * Dk).astype(f32)                 # [K, 6]
    corr = (wk @ (E_SYN * a.sum(axis=1, dtype=f32))).astype(f32)  # [K]

    def dev(x):  # [N] -> [P, Q] with j = ((q + QSH*c) % Q)*128 + p
        m = np.asarray(x, f32).reshape(Q, P).T          # [P, Q]
        return np.ascontiguousarray(np.roll(m, -QSH * c, axis=1))

    ie2 = dev(DT * np.asarray(I_ext, f32) + DT * G_L * E_L)
    v0 = dev(V0)
    g0 = np.concatenate([dev(m0), dev(h0), dev(n0)], axis=1)      # [P, 3Q]
    a_rot = np.roll(a.reshape(6, Q, P), -QSH * c, axis=1).reshape(6, N)
    return dict(
        a6=np.ascontiguousarray(a_rot), wkT=np.ascontiguousarray(wk.T),
        corr=corr[None, :], ie2=ie2, v0=v0, g0=np.ascontiguousarray(g0),
    )


def kernel(I_ext, synaptic_weights, V0, m0, h0, n0, s0, synapse_types, T):
    f32 = np.float32
    W = np.asarray(synaptic_weights, f32)
    ty = np.ascontiguousarray(np.asarray(synapse_types, np.int32))
    num_steps = int(int(T) / DT)
    core_ids = list(range(NCORES))

    # launch A: sharded masked column sums
    nc_a = build_cols_kernel()
    nc_a.finalize()
    in_maps_a = [
        {"w": np.ascontiguousarray(W[:, c * NSH:(c + 1) * NSH]),
         "ty": np.ascontiguousarray(ty[:, c * NSH:(c + 1) * NSH])}
        for c in core_ids
    ]
    res_a = run_bass_kernel_spmd(nc_a, in_maps_a, core_ids).results
    cols = np.concatenate([res_a[c]["cols"] for c in core_ids], axis=1)  # [6, N]

    # host glue: tiny vectors only, per-core rotated
    in_maps_b = [
        _host_prep(I_ext, V0, m0, h0, n0, s0, cols, num_steps, c)
        for c in core_ids
    ]

    # launch B: replicated scan
    nc_b = build_scan_kernel(num_steps)
    nc_b.finalize()
    res_b = run_bass_kernel_spmd(nc_b, in_maps_b, core_ids).results

    # assemble: core c wrote V[:, ds(4c, 4)] -> [K, 128, 4] in (p, qq) order;
    # neuron j = (4c+qq)*128 + p
    out = np.empty((num_steps, N), f32)
    for c in core_ids:
        part = res_b[c]["vh"].reshape(num_steps, P, QSH)
        out[:, c * P * QSH:(c + 1) * P * QSH] = (
            part.transpose(0, 2, 1).reshape(num_steps, P * QSH)
        )
    return out
